# revision 1
# baseline (speedup 1.0000x reference)
"""Linformer self-attention on 8 Trainium2 NeuronCores.

Sharding: core = (batch b, head-group g) with b = core//2, g = core%2.
Each core computes attention for batch b and its 8 heads (512 of the 1024
channels), then a row-sharded W_out matmul producing a partial output in
transposed [1024, 4096] layout; the host sums the two partials per batch,
transposes once, and adds b_out.

Per-core dataflow (all matmuls on TensorE; out = lhsT.T @ rhs):
  A: k = x@Wk, v = x@Wv streamed in 512-row slabs (from host-transposed xT),
     accumulated into kpT = (E^T k)^T laid out [c, kr] and vp = E^T v [kr, c].
  B: per slab: qT = Wq^T x^T; per head: natural scores (f32r) -> true row max
     (negated, PE-transposed to a row), scores^T via matmul with a K=1
     ones x (-max) correction row, exp on ScalarE -> U^T (fp16, <=1),
     denominators via ones-matmul (PE broadcast to all 128 partitions),
     reciprocal, AV matmul, normalization fused into the PSUM eviction.
  C: out_partial^T = Wo-chunk-stationary matmuls over attn_out^T.

Precision: q/k/score chain in float32r (~13 significand bits, full PE rate);
v chain f32r until vp, then fp16 (U^T, vp, attn_out^T, W_out).
"""

import os
import numpy as np

import concourse.bacc as bacc
import concourse.tile as tile
from concourse import mybir
from concourse.bass_utils import run_bass_kernel_spmd

F32 = mybir.dt.float32
F16 = mybir.dt.float16
F32R = mybir.dt.float32r
EXP = mybir.ActivationFunctionType.Exp
AXX = mybir.AxisListType.X

DIM, SEQ, KR, HD = 1024, 4096, 256, 64
CG = 512               # channels per head-group (8 heads x 64)
NSLAB = 512
SLABS = SEQ // NSLAB   # 8
DC = DIM // 128        # 8 contraction chunks over d
SCALE = HD ** -0.5

_cache = {}


def build_program():
    nc = bacc.Bacc("TRN2", target_bir_lowering=False, debug=False, num_devices=8)

    xT = nc.dram_tensor("xT", [DC, 128, SEQ], F32R, kind="ExternalInput")
    Wq = nc.dram_tensor("Wq", [DC, 128, CG], F32R, kind="ExternalInput")
    Wk = nc.dram_tensor("Wk", [DC, 128, CG], F32R, kind="ExternalInput")
    Wv = nc.dram_tensor("Wv", [DC, 128, CG], F32R, kind="ExternalInput")
    Ed = nc.dram_tensor("E", [SLABS, 4, 128, KR], F32R, kind="ExternalInput")
    Wo = nc.dram_tensor("Wo", [CG // 128, 128, DIM], F16, kind="ExternalInput")
    ident = nc.dram_tensor("ident", [128, 128], F32, kind="ExternalInput")
    ones1_d = nc.dram_tensor("ones1", [1, 128], F32R, kind="ExternalInput")
    out_d = nc.dram_tensor("out", [DIM, SEQ], F32, kind="ExternalOutput")

    mm = nc.tensor.matmul

    with tile.TileContext(nc) as tc:
        with tc.tile_pool(name="const", bufs=1) as const:
            kpT_sb = const.tile([128, 4, KR], F32R)  # (E^T k)^T: [c, kr], c = p + 128*ct
            vp_sb = const.tile([128, 2, CG], F16)    # E^T v: [kr, c], kr = p + 128*krt
            outU = const.tile([128, 4, SEQ], F16)    # attn_out^T: [c, n]
            id_sb = const.tile([128, 128], F32)
            ones1 = const.tile([1, 128], F32R)
            ones128 = const.tile([128, 128], F16)
            wq_sb = const.tile([128, DC, CG], F32R)
            wo_sb = const.tile([128, 4, DIM], F16)

            # ---------------- Phase A: k, v -> kpT, vp ----------------
            with tc.tile_pool(name="pA", bufs=1) as pA, \
                 tc.tile_pool(name="psA", bufs=1, space="PSUM") as psA:
                # slab-0 x DMAs issued first so the PE can start ASAP
                xs0 = pA.tile([128, DC, NSLAB], F32R, tag="xs", bufs=2)
                for dc in range(DC):
                    nc.sync.dma_start(out=xs0[:, dc, :], in_=xT[dc, :, 0:NSLAB])
                wk_sb = pA.tile([128, DC, CG], F32R)
                for dc in range(DC):
                    nc.sync.dma_start(out=wk_sb[:, dc, :], in_=Wk[dc])
                wv_sb = pA.tile([128, DC, CG], F32R)
                for dc in range(DC):
                    nc.sync.dma_start(out=wv_sb[:, dc, :], in_=Wv[dc])
                nc.sync.dma_start(out=id_sb, in_=ident[:, :])
                nc.sync.dma_start(out=ones1, in_=ones1_d[:, :])
                nc.vector.memset(ones128, 1.0)
                for dc in range(DC):
                    nc.sync.dma_start(out=wq_sb[:, dc, :], in_=Wq[dc])
                for ct in range(4):
                    nc.sync.dma_start(out=wo_sb[:, ct, :], in_=Wo[ct])

                kpT_ps = psA.tile([128, 4, 512], F32)  # one bank per ct accum group
                vp_ps = psA.tile([128, 2, CG], F32)
                for s in range(SLABS):
                    if s == 0:
                        xs = xs0
                    else:
                        xs = pA.tile([128, DC, NSLAB], F32R, tag="xs", bufs=2)
                        for dc in range(DC):
                            nc.sync.dma_start(
                                out=xs[:, dc, :], in_=xT[dc, :, s * NSLAB:(s + 1) * NSLAB])
                    e_sb = pA.tile([128, 4, KR], F32R, tag="esb", bufs=2)
                    for ns in range(4):
                        nc.sync.dma_start(out=e_sb[:, ns, :], in_=Ed[s, ns])
                    kslab = pA.tile([128, 4, CG], F32R, tag="kslab", bufs=2)
                    vslab = pA.tile([128, 4, CG], F32R, tag="vslab", bufs=2)
                    first, last = (s == 0), (s == SLABS - 1)
                    for ns in range(4):
                        k_ps = psA.tile([128, CG], F32, tag="kvps", bufs=2)
                        for dc in range(DC):
                            mm(k_ps, lhsT=xs[:, dc, ns * 128:(ns + 1) * 128],
                               rhs=wk_sb[:, dc, :], start=(dc == 0), stop=(dc == DC - 1))
                        nc.scalar.copy(kslab[:, ns, :], k_ps)
                        v_ps = psA.tile([128, CG], F32, tag="kvps", bufs=2)
                        for dc in range(DC):
                            mm(v_ps, lhsT=xs[:, dc, ns * 128:(ns + 1) * 128],
                               rhs=wv_sb[:, dc, :], start=(dc == 0), stop=(dc == DC - 1))
                        nc.scalar.copy(vslab[:, ns, :], v_ps)
                        for ct in range(4):
                            mm(kpT_ps[:, ct, 0:KR],
                               lhsT=kslab[:, ns, ct * 128:(ct + 1) * 128],
                               rhs=e_sb[:, ns, :],
                               start=(first and ns == 0), stop=(last and ns == 3))
                        for krt in range(2):
                            mm(vp_ps[:, krt, :],
                               lhsT=e_sb[:, ns, krt * 128:(krt + 1) * 128],
                               rhs=vslab[:, ns, :],
                               start=(first and ns == 0), stop=(last and ns == 3))
                nc.vector.tensor_copy(kpT_sb, kpT_ps[:, :, 0:KR])
                nc.vector.tensor_copy(vp_sb, vp_ps)

            # ---------------- Phase B: qT, scores, softmax, AV ----------------
            with tc.tile_pool(name="pB", bufs=1) as pB, \
                 tc.tile_pool(name="psB", bufs=1, space="PSUM") as psB:
                for s in range(SLABS):
                    xs = pB.tile([128, DC, NSLAB], F32R, tag="xs", bufs=2)
                    for dc in range(DC):
                        nc.sync.dma_start(
                            out=xs[:, dc, :], in_=xT[dc, :, s * NSLAB:(s + 1) * NSLAB])
                    qt = pB.tile([128, 4, NSLAB], F32R, tag="qt", bufs=2)
                    for ct in range(4):
                        q_ps = psB.tile([128, NSLAB], F32, tag="st", bufs=2)
                        for dc in range(DC):
                            mm(q_ps, lhsT=wq_sb[:, dc, ct * 128:(ct + 1) * 128],
                               rhs=xs[:, dc, :], start=(dc == 0), stop=(dc == DC - 1))
                        nc.scalar.copy(qt[:, ct, :], q_ps)
                    def stage_nat(h, qh, kph):
                        natA = psB.tile([128, 2, KR], F32, tag="nat", bufs=2,
                                        name=f"natA_{s}_{h}")
                        natB = psB.tile([128, 2, KR], F32, tag="nat", bufs=2,
                                        name=f"natB_{s}_{h}")
                        for half, natp in ((0, natA), (1, natB)):
                            for i in range(2):
                                ns = half * 2 + i
                                mm(natp[:, i, :],
                                   lhsT=qh[:, ns * 128:(ns + 1) * 128],
                                   rhs=kph, start=(i == 0), stop=(i == 1))
                        mrows = pB.tile([128, 4], F32, tag="mrows", bufs=3,
                                        name=f"mrows_{s}_{h}")
                        nc.vector.reduce_max(mrows[:, 0:2], natA, axis=AXX, negate=True)
                        nc.vector.reduce_max(mrows[:, 2:4], natB, axis=AXX, negate=True)
                        return mrows

                    def stage_negm(h, mrows):
                        negm_ps = psB.tile([1, NSLAB], F32, tag="negm_ps", bufs=1,
                                           name=f"negmps_{s}_{h}")
                        for ns in range(4):
                            mm(negm_ps[0:1, ns * 128:(ns + 1) * 128],
                               lhsT=mrows[:, ns:ns + 1], rhs=id_sb, is_transpose=True,
                               start=(ns == 0), stop=(ns == 3))
                        negm = pB.tile([1, NSLAB], F32R, tag="negm", bufs=2,
                                       name=f"negm_{s}_{h}")
                        nc.scalar.copy(negm, negm_ps)
                        return negm

                    def stage_scores(h, qh, kph, negm):
                        U = pB.tile([128, 2, NSLAB], F16, tag="U", bufs=2,
                                    name=f"U_{s}_{h}")
                        for krt in range(2):
                            st_ps = psB.tile([128, NSLAB], F32, tag="st", bufs=2,
                                             name=f"st_{s}_{h}_{krt}")
                            mm(st_ps, lhsT=kph[:, krt * 128:(krt + 1) * 128],
                               rhs=qh, start=True, stop=False)
                            mm(st_ps, lhsT=ones1, rhs=negm, start=False, stop=True)
                            nc.scalar.activation(U[:, krt, :], st_ps, EXP,
                                                 bias=0.0, scale=SCALE)
                        return U

                    def stage_av(h, hp, ct_h, U):
                        sum_ps = psB.tile([128, NSLAB], F32, tag="sums", bufs=2,
                                          name=f"sum_{s}_{h}")
                        for krt in range(2):
                            mm(sum_ps, lhsT=ones128, rhs=U[:, krt, :],
                               start=(krt == 0), stop=(krt == 1))
                        srecip = pB.tile([128, NSLAB], F32, tag="srecip", bufs=2,
                                         name=f"srecip_{s}_{h}")
                        nc.vector.reciprocal(srecip, sum_ps)
                        av_ps = psB.tile([128, NSLAB], F32, tag="av", bufs=1,
                                         name=f"av_{s}_{h}")
                        for krt in range(2):
                            mm(av_ps[hp:hp + 64, :],
                               lhsT=vp_sb[:, krt, h * 64:(h + 1) * 64],
                               rhs=U[:, krt, :], start=(krt == 0), stop=(krt == 1))
                        nc.vector.tensor_mul(
                            outU[hp:hp + 64, ct_h, s * NSLAB:(s + 1) * NSLAB],
                            av_ps[hp:hp + 64, :], srecip[hp:hp + 64, :])

                    # two-stage software pipeline over heads
                    prev = None
                    for h in range(8):
                        hp = (h % 2) * 64
                        ct_h = h // 2
                        qh = qt[hp:hp + 64, ct_h, :]
                        kph = kpT_sb[hp:hp + 64, ct_h, :]
                        mrows = stage_nat(h, qh, kph)
                        if prev is not None:
                            pU = stage_scores(prev[0], prev[1], prev[2], prev[3])
                        negm = stage_negm(h, mrows)
                        if prev is not None:
                            stage_av(prev[0], prev[4], prev[5], pU)
                        prev = (h, qh, kph, negm, hp, ct_h)
                    pU = stage_scores(prev[0], prev[1], prev[2], prev[3])
                    stage_av(prev[0], prev[4], prev[5], pU)
                    # ---- phase C streamed per slab: out^T = Wo-stationary @ attn_out^T ----
                    for jc in range(DIM // 128):
                        f_ps = psB.tile([128, 512], F32, tag="sums", bufs=2,
                                        name=f"fps_{s}_{jc}")
                        for ct in range(4):
                            mm(f_ps, lhsT=wo_sb[:, ct, jc * 128:(jc + 1) * 128],
                               rhs=outU[:, ct, s * NSLAB:(s + 1) * NSLAB],
                               start=(ct == 0), stop=(ct == 3))
                        ot = pB.tile([128, 512], F32, tag="ot", bufs=6)
                        if jc % 2 == 0:
                            nc.vector.tensor_copy(ot, f_ps)
                        else:
                            nc.scalar.copy(ot, f_ps)
                        nc.sync.dma_start(
                            out=out_d[jc * 128:(jc + 1) * 128, s * NSLAB:(s + 1) * NSLAB],
                            in_=ot)

    nc.compile()
    return nc


def kernel(x, W_qkv, E, W_out, b_out):
    x = np.ascontiguousarray(np.asarray(x, dtype=np.float32))
    W_qkv = np.asarray(W_qkv, dtype=np.float32)
    E_np = np.asarray(E, dtype=np.float32)
    W_out = np.asarray(W_out, dtype=np.float32)
    b_out = np.asarray(b_out, dtype=np.float32)

    if "nc" not in _cache:
        _cache["nc"] = build_program()
    nc = _cache["nc"]

    E_t = np.ascontiguousarray(E_np.reshape(SLABS, 4, 128, KR))
    ident = np.eye(128, dtype=np.float32)
    ones_row = np.ones((1, 128), dtype=np.float32)
    in_maps = []
    for core in range(8):
        b, g = core // 2, core % 2
        cols = slice(g * CG, (g + 1) * CG)
        xT_t = np.ascontiguousarray(x[b].T).reshape(DC, 128, SEQ)
        Wq_t = np.ascontiguousarray(W_qkv[:, 0 * DIM:1 * DIM][:, cols]).reshape(DC, 128, CG)
        Wk_t = np.ascontiguousarray(W_qkv[:, 1 * DIM:2 * DIM][:, cols]).reshape(DC, 128, CG)
        Wv_t = np.ascontiguousarray(W_qkv[:, 2 * DIM:3 * DIM][:, cols]).reshape(DC, 128, CG)
        Wo_t = np.ascontiguousarray(W_out[g * CG:(g + 1) * CG, :].astype(np.float16)).reshape(
            CG // 128, 128, DIM)
        in_maps.append({
            "xT": xT_t, "Wq": Wq_t, "Wk": Wk_t, "Wv": Wv_t,
            "E": E_t, "Wo": Wo_t, "ident": ident, "ones1": ones_row,
        })

    trace = bool(int(os.environ.get("KERNEL_TRACE", "0")))
    res = run_bass_kernel_spmd(nc, in_maps, core_ids=list(range(8)), trace=trace)
    _cache["last_results"] = res

    # partials come back transposed [DIM, SEQ]; sum per batch, transpose once
    accT = np.zeros((4, DIM, SEQ), dtype=np.float32)
    for core in range(8):
        accT[core // 2] += res.results[core]["out"]
    out = np.ascontiguousarray(accT.transpose(0, 2, 1))
    out += b_out[None, None, :]
    return out

